# revision 1
# baseline (speedup 1.0000x reference)
"""Trainium2 Bass kernel for nn_BDH_39127152067244 (dense_transformer).

Sharding: 8 cores = (b, h) pairs — b = core // 4, h = core % 4. Each core
computes its head's share of every layer; the only cross-core communication
is a 4-rank AllReduce of the per-head yMLP partial [T, D] once per layer
(replica groups {0..3} and {4..7}).

Layout tricks:
  - The N axis (8192) is deinterleaved on the host (even n first, odd n
    second), applied consistently to encoder / encoder_v / decoder rows and
    the rope tables. Rope's interleaved pair-swap then becomes a clean
    half-offset of whole 128-partition tiles with a sign folded into the
    sin table.
  - x_sparse is computed directly in transposed [N, T] layout (encoder is
    already the right lhsT layout), which is what both sides of the scores
    Gram matmul and the decoder matmul want.
  - scores: the Gram matrix of rope'd activations is symmetric, so the
    strict-lower-triangular masked scores in [t, s] layout equal the
    strict-upper masked Gram in [s, t] layout — computed directly as the
    yKV matmul's lhsT. Fully-masked tiles are never computed.
  - All matmuls run in bf16 with f32 PSUM accumulation; LayerNorms and the
    residual stream stay f32.
"""

import math
import sys
from contextlib import ExitStack

import numpy as np
import ml_dtypes

sys.path.insert(0, "/opt/trn_rl_repo")

import concourse.bass as bass  # noqa: E402
import concourse.bacc as bacc  # noqa: E402
import concourse.mybir as mybir  # noqa: E402
import concourse.tile as tile  # noqa: E402
from concourse.bass import ds  # noqa: E402
from concourse.bass_utils import run_bass_kernel_spmd  # noqa: E402
from concourse.masks import make_identity  # noqa: E402

BF16 = ml_dtypes.bfloat16
BF = mybir.dt.bfloat16
FP32 = mybir.dt.float32
AF = mybir.ActivationFunctionType
ALU = mybir.AluOpType

# Problem constants (hardcoded per the harness contract).
N_LAYER = 6
D = 256
NH = 4
N = 8192
HALF = N // 2
VOCAB = 256
B, T = 2, 512
THETA = 2.0**16
EPS = 1e-5

P = 128          # partitions
NT = N // P      # 64 n-tiles
G4 = 4           # n-tiles per rope/qx group
NG = NT // G4    # 16 groups
VG = 8           # n-tiles per V tile
NVG = NT // VG   # 8 V tiles
TC = T // P      # 4 t-chunks
DT = D // P      # 2 d-tiles
N_CORES = 8

_CACHE: dict = {}


def _build_bass():
    nc = bacc.Bacc("TRN2", num_devices=N_CORES)

    x0_d = nc.dram_tensor("x0", [P, TC, D], FP32, kind="ExternalInput")
    x0bf_d = nc.dram_tensor("x0bf", [P, TC, D], BF, kind="ExternalInput")
    x0T_d = nc.dram_tensor("x0T", [P, DT, T], BF, kind="ExternalInput")
    enc_d = nc.dram_tensor("enc", [DT, P, NT, P], BF, kind="ExternalInput")
    encv_d = nc.dram_tensor("encv", [DT, P, NT, P], BF, kind="ExternalInput")
    dec_d = nc.dram_tensor("dec", [P, NT, D], BF, kind="ExternalInput")
    cos_d = nc.dram_tensor("cosb", [P, NT, T], BF, kind="ExternalInput")
    sin_d = nc.dram_tensor("sinb", [P, NT, T], BF, kind="ExternalInput")
    mask_d = nc.dram_tensor("maskb", [P, TC, T], BF, kind="ExternalInput")
    lm_d = nc.dram_tensor("lm", [P, DT, VOCAB], BF, kind="ExternalInput")
    out_d = nc.dram_tensor("logits", [P, TC, VOCAB], FP32, kind="ExternalOutput")

    with tile.TileContext(nc) as tc, ExitStack() as ctx:
        sb = ctx.enter_context(tc.tile_pool(name="sb", bufs=1))
        vpool = ctx.enter_context(tc.tile_pool(name="vpool", bufs=NVG))
        qxpool = ctx.enter_context(tc.tile_pool(name="qxpool", bufs=5))
        wpool = ctx.enter_context(tc.tile_pool(name="wpool", bufs=2))
        tabpool = ctx.enter_context(tc.tile_pool(name="tabpool", bufs=2))
        roppool = ctx.enter_context(tc.tile_pool(name="roppool", bufs=2))
        mixpool = ctx.enter_context(tc.tile_pool(name="mixpool", bufs=2))
        statpool = ctx.enter_context(tc.tile_pool(name="statpool", bufs=8))
        xpool = ctx.enter_context(tc.tile_pool(name="xpool", bufs=2))
        apsum = ctx.enter_context(tc.tile_pool(name="apsum", bufs=2, space="PSUM"))
        cpsum = ctx.enter_context(tc.tile_pool(name="cpsum", bufs=1, space="PSUM"))
        drm = ctx.enter_context(tc.tile_pool(name="drm", bufs=2, space="DRAM"))

        ident = sb.tile([P, P], BF, name="ident")
        make_identity(nc, ident)
        epst = sb.tile([P, 1], FP32, name="epst")
        nc.vector.memset(epst, EPS)
        maskt = sb.tile([P, TC, T], BF, name="maskt")
        nc.sync.dma_start(out=maskt, in_=mask_d[:])
        lmt = sb.tile([P, DT, VOCAB], BF, name="lmt")
        nc.sync.dma_start(out=lmt, in_=lm_d[:])

        x_f = xpool.tile([P, TC, D], FP32, tag="xf", name="x_f0")
        nc.sync.dma_start(out=x_f, in_=x0_d[:])
        x_bf = xpool.tile([P, TC, D], BF, tag="xbf", name="x_bf0")
        nc.sync.dma_start(out=x_bf, in_=x0bf_d[:])
        x_T = xpool.tile([P, DT, T], BF, tag="xT", name="x_T0")
        nc.sync.dma_start(out=x_T, in_=x0T_d[:])

        def layer_norm_stats(src_ap, name):
            """Returns (mv, rstd) where mv[:,0:1]=mean, rstd=1/sqrt(var+eps)."""
            stats = statpool.tile([P, 6], FP32, tag="bst", name=f"st_{name}")
            nc.vector.bn_stats(out=stats, in_=src_ap)
            mv = statpool.tile([P, 2], FP32, tag="bmv", name=f"mv_{name}")
            nc.vector.bn_aggr(out=mv, in_=stats)
            rstd = statpool.tile([P, 1], FP32, tag="brs", name=f"rs_{name}")
            nc.scalar.activation(out=rstd, in_=mv[:, 1:2], func=AF.Sqrt, bias=epst)
            nc.vector.reciprocal(rstd, rstd)
            return mv, rstd

        def emit_layer(l, x_f, x_bf, x_T):
            # ---------------- step A: V^T = relu(enc^T @ x^T), [N, T] ------
            V = [None] * NVG

            def emit_A(vg):
                encg = wpool.tile([P, DT, VG, P], BF, tag="w", name=f"enc{l}_{vg}")
                nc.sync.dma_start(
                    out=encg,
                    in_=enc_d[:, :, ds(vg * VG, VG), :].rearrange(
                        "dt p nt n -> p dt nt n"
                    ),
                )
                vt = vpool.tile([P, VG, T], BF, tag="v", name=f"v{l}_{vg}")
                V[vg] = vt
                for q in range(VG // 2):
                    ps = apsum.tile(
                        [P, 2, T], FP32, tag="quad", name=f"aps{l}_{vg}_{q}"
                    )
                    for i in range(2):
                        for dt_ in range(DT):
                            nc.tensor.matmul(
                                ps[:, i, :],
                                lhsT=encg[:, dt_, q * 2 + i, :],
                                rhs=x_T[:, dt_, :],
                                start=(dt_ == 0),
                                stop=(dt_ == DT - 1),
                            )
                    nc.scalar.activation(
                        out=vt[:, ds(q * 2, 2), :], in_=ps, func=AF.Relu
                    )

            # ---------------- rope: QR = V*cos + Vpartner*sin' -------------
            QR = [None] * NG

            def emit_rope(g):
                cosg = tabpool.tile([P, G4, T], BF, tag="cos", name=f"cos{l}_{g}")
                nc.sync.dma_start(out=cosg, in_=cos_d[:, ds(g * G4, G4), :])
                sing = tabpool.tile([P, G4, T], BF, tag="sin", name=f"sin{l}_{g}")
                nc.sync.dma_start(out=sing, in_=sin_d[:, ds(g * G4, G4), :])
                qr = qxpool.tile([P, G4, T], BF, tag="qx", name=f"qr{l}_{g}")
                QR[g] = qr
                pg = roppool.tile([P, G4, T], BF, tag="rp", name=f"rp{l}_{g}")
                p2 = roppool.tile([P, G4, T], BF, tag="rp2", name=f"rq{l}_{g}")
                vg_, off = divmod(g * G4, VG)
                pvg_, poff = divmod((g ^ (NG // 2)) * G4, VG)
                nc.vector.tensor_mul(pg, V[vg_][:, ds(off, G4), :], cosg)
                nc.vector.tensor_mul(p2, V[pvg_][:, ds(poff, G4), :], sing)
                nc.vector.tensor_add(qr, pg, p2)

            for pair in range(NVG // 2):
                emit_A(pair)
                emit_A(pair + NVG // 2)
                emit_rope(pair * 2)
                emit_rope(pair * 2 + 1)
            for g in range(NG // 2, NG):
                emit_rope(g)

            # ---------------- step C: masked Gram in [s, t] ----------------
            gps = cpsum.tile([P, TC, T], FP32, tag="mm", name=f"gps{l}")
            for k in range(NT):
                g, i = divmod(k, G4)
                for j in range(TC):
                    nc.tensor.matmul(
                        gps[:, j, : T - j * P],
                        lhsT=QR[g][:, i, ds(j * P, P)],
                        rhs=QR[g][:, i, ds(j * P, T - j * P)],
                        start=(k == 0),
                        stop=(k == NT - 1),
                    )
            st = mixpool.tile([P, TC, T], BF, tag="st", name=f"st{l}")
            for j in range(TC):
                nc.vector.tensor_mul(
                    st[:, j, ds(j * P, T - j * P)],
                    gps[:, j, : T - j * P],
                    maskt[:, j, ds(j * P, T - j * P)],
                )

            # ---------------- step D: yKV = M^T @ x, then LN ---------------
            dps = cpsum.tile([P, TC, T], FP32, tag="mm", name=f"dps{l}")
            for jp in range(TC):
                for i in range(jp + 1):
                    nc.tensor.matmul(
                        dps[:, jp, :D],
                        lhsT=st[:, i, ds(jp * P, P)],
                        rhs=x_bf[:, i, :],
                        start=(i == 0),
                        stop=(i == jp),
                    )
            yln = mixpool.tile([P, TC, D], BF, tag="yln", name=f"yln{l}")
            for jp in range(TC):
                mv, rstd = layer_norm_stats(dps[:, jp, :D], f"d{l}_{jp}")
                nc.vector.tensor_scalar(
                    out=yln[:, jp, :],
                    in0=dps[:, jp, :D],
                    scalar1=mv[:, 0:1],
                    scalar2=rstd,
                    op0=ALU.subtract,
                    op1=ALU.mult,
                )
            ylnT = mixpool.tile([P, DT, T], BF, tag="ylnT", name=f"ylnT{l}")
            for dt_ in range(DT):
                tp = apsum.tile([P, TC, P], BF, tag="quad", name=f"ytp{l}_{dt_}")
                for jp in range(TC):
                    nc.tensor.transpose(
                        tp[:, jp, :], yln[:, jp, ds(dt_ * P, P)], ident
                    )
                nc.scalar.copy(
                    out=ylnT[:, dt_, :].rearrange("p (a b) -> p a b", a=TC),
                    in_=tp,
                )

            # ---------------- step E: gated y_sparse, [N, T] ---------------
            XY = [None] * NG
            for vg in range(NVG):
                evg = wpool.tile([P, DT, VG, P], BF, tag="w", name=f"ev{l}_{vg}")
                nc.sync.dma_start(
                    out=evg,
                    in_=encv_d[:, :, ds(vg * VG, VG), :].rearrange(
                        "dt p nt n -> p dt nt n"
                    ),
                )
                for half in range(2):
                    g = vg * 2 + half
                    xy = qxpool.tile([P, G4, T], BF, tag="qx", name=f"xy{l}_{g}")
                    XY[g] = xy
                    for q in range(2):
                        ps = apsum.tile(
                            [P, 2, T], FP32, tag="quad", name=f"eps{l}_{g}_{q}"
                        )
                        for i in range(2):
                            nt_ = half * G4 + q * 2 + i
                            for dt_ in range(DT):
                                nc.tensor.matmul(
                                    ps[:, i, :],
                                    lhsT=evg[:, dt_, nt_, :],
                                    rhs=ylnT[:, dt_, :],
                                    start=(dt_ == 0),
                                    stop=(dt_ == DT - 1),
                                )
                        ys = roppool.tile(
                            [P, 2, T], BF, tag="rp2", name=f"ys{l}_{g}_{q}"
                        )
                        nc.scalar.activation(out=ys, in_=ps, func=AF.Relu)
                        nc.vector.tensor_mul(
                            xy[:, ds(q * 2, 2), :],
                            ys,
                            V[vg][:, ds(half * G4 + q * 2, 2), :],
                        )

            # ---------------- step F: yMLP partial = XY^T @ dec ------------
            fps = cpsum.tile([P, TC, T], FP32, tag="mm", name=f"fps{l}")
            for g in range(NG):
                decg = wpool.tile([P, G4, D], BF, tag="dec", name=f"dec{l}_{g}")
                nc.sync.dma_start(out=decg, in_=dec_d[:, ds(g * G4, G4), :])
                for i in range(G4):
                    k = g * G4 + i
                    for m in range(TC):
                        nc.tensor.matmul(
                            fps[:, m, :D],
                            lhsT=XY[g][:, i, ds(m * P, P)],
                            rhs=decg[:, i, :],
                            start=(k == 0),
                            stop=(k == NT - 1),
                        )

            # ---------------- AllReduce over the 4 heads of this batch ----
            ymlp = mixpool.tile([P, TC, D], BF, tag="ym", name=f"ym{l}")
            ymr = mixpool.tile([P, TC, D], BF, tag="ymr", name=f"ymr{l}")
            for hv in range(2):
                nc.scalar.copy(
                    out=ymlp[:, ds(hv * 2, 2), :],
                    in_=fps[:, ds(hv * 2, 2), :D],
                )
                cc_in = drm.tile(
                    [P, 2, D], BF, tag=f"ccin{hv}", name=f"ccin{l}_{hv}"
                )
                cc_out = drm.tile(
                    [P, 2, D], BF, tag=f"ccout{hv}", name=f"ccout{l}_{hv}"
                )
                nc.sync.dma_start(out=cc_in[:], in_=ymlp[:, ds(hv * 2, 2), :])
                nc.gpsimd.collective_compute(
                    "AllReduce",
                    ALU.add,
                    replica_groups=[[0, 1, 2, 3], [4, 5, 6, 7]],
                    ins=[cc_in[:]],
                    outs=[cc_out[:]],
                )
                nc.sync.dma_start(out=ymr[:, ds(hv * 2, 2), :], in_=cc_out[:])

            # ---------------- x = LN(x + LN(yMLP)) -------------------------
            x_f_new = xpool.tile([P, TC, D], FP32, tag="xf", name=f"x_f{l + 1}")
            x_bf_new = xpool.tile([P, TC, D], BF, tag="xbf", name=f"x_bf{l + 1}")
            xmid = mixpool.tile([P, TC, D], FP32, tag="xmid", name=f"xm{l}")
            for jp in range(TC):
                mv1, r1 = layer_norm_stats(ymr[:, jp, :], f"y{l}_{jp}")
                nc.vector.scalar_tensor_tensor(
                    out=xmid[:, jp, :],
                    in0=ymr[:, jp, :],
                    scalar=r1,
                    in1=x_f[:, jp, :],
                    op0=ALU.mult,
                    op1=ALU.add,
                )
                mv2, r2 = layer_norm_stats(xmid[:, jp, :], f"x{l}_{jp}")
                nc.vector.tensor_scalar(
                    out=x_bf_new[:, jp, :],
                    in0=xmid[:, jp, :],
                    scalar1=mv2[:, 0:1],
                    scalar2=r2,
                    op0=ALU.subtract,
                    op1=ALU.mult,
                )
                nc.vector.tensor_scalar(
                    out=x_f_new[:, jp, :],
                    in0=xmid[:, jp, :],
                    scalar1=mv2[:, 0:1],
                    scalar2=r2,
                    op0=ALU.subtract,
                    op1=ALU.mult,
                )
            x_T_new = xpool.tile([P, DT, T], BF, tag="xT", name=f"x_T{l + 1}")
            for dt_ in range(DT):
                tp = apsum.tile([P, TC, P], BF, tag="quad", name=f"xtp{l}_{dt_}")
                for jp in range(TC):
                    nc.tensor.transpose(
                        tp[:, jp, :], x_bf_new[:, jp, ds(dt_ * P, P)], ident
                    )
                nc.scalar.copy(
                    out=x_T_new[:, dt_, :].rearrange("p (a b) -> p a b", a=TC),
                    in_=tp,
                )
            return x_f_new, x_bf_new, x_T_new

        for l in range(N_LAYER):
            x_f, x_bf, x_T = emit_layer(l, x_f, x_bf, x_T)

        # ---------------- lm head -----------------------------------------
        lps = cpsum.tile([P, TC, T], FP32, tag="mm", name="lps")
        for jp in range(TC):
            for dt_ in range(DT):
                nc.tensor.matmul(
                    lps[:, jp, :VOCAB],
                    lhsT=x_T[:, dt_, ds(jp * P, P)],
                    rhs=lmt[:, dt_, :],
                    start=(dt_ == 0),
                    stop=(dt_ == DT - 1),
                )
        lout = mixpool.tile([P, TC, VOCAB], FP32, tag="lout", name="lout")
        nc.scalar.copy(out=lout, in_=lps[:, :, :VOCAB])
        nc.sync.dma_start(out=out_d[:], in_=lout)

    if not nc.is_finalized():
        nc.finalize()
    return nc


def _ln_np(x):
    m = x.mean(-1, keepdims=True)
    v = ((x - m) ** 2).mean(-1, keepdims=True)
    return (x - m) / np.sqrt(v + EPS)


def _make_tables():
    t = np.arange(N, dtype=np.float32)
    q = np.floor(t / 2.0) * 2.0
    freqs = (1.0 / (THETA ** (q / N)) / (2.0 * np.float32(math.pi))).astype(
        np.float32
    )
    phases = np.arange(T, dtype=np.float32)[:, None] * freqs[None, :]
    ph = np.float32(np.float32(phases % 1.0) * np.float32(2.0 * math.pi))
    return np.cos(ph).astype(np.float32), np.sin(ph).astype(np.float32)


def _prep_inputs(idx, embed_w, encoder, encoder_v, decoder, lm_head):
    perm = np.concatenate([np.arange(HALF) * 2, np.arange(HALF) * 2 + 1])

    cos, sin = _make_tables()
    cosp = cos[:, perm]
    sinp = sin[:, perm].copy()
    sinp[:, :HALF] *= -1.0
    # [P, NT, T]: (p, nt, t) -> table[t, nt*P + p]
    cos_h = np.ascontiguousarray(
        cosp.T.reshape(NT, P, T).transpose(1, 0, 2)
    ).astype(BF16)
    sin_h = np.ascontiguousarray(
        sinp.T.reshape(NT, P, T).transpose(1, 0, 2)
    ).astype(BF16)

    mask_h = np.zeros((P, TC, T), np.float32)
    t_idx = np.arange(T)
    for j in range(TC):
        for p in range(P):
            mask_h[p, j] = (t_idx > (j * P + p)).astype(np.float32)
    mask_h = mask_h.astype(BF16)

    lm_h = np.ascontiguousarray(
        lm_head.reshape(DT, P, VOCAB).transpose(1, 0, 2)
    ).astype(BF16)

    x0 = _ln_np(embed_w[idx].astype(np.float32))  # (B, T, D)

    dec3 = decoder.reshape(NH, N, D)

    per_core = []
    for core in range(N_CORES):
        b, h = divmod(core, NH)
        enc_p = encoder[h][:, perm]  # (D, N)
        encv_p = encoder_v[h][:, perm]
        dec_p = dec3[h][perm, :]  # (N, D)

        enc_h = enc_p.reshape(DT, P, NT, P).astype(BF16)
        encv_h = encv_p.reshape(DT, P, NT, P).astype(BF16)
        dec_h = np.ascontiguousarray(
            dec_p.reshape(NT, P, D).transpose(1, 0, 2)
        ).astype(BF16)

        xb = x0[b]  # (T, D) f32
        x0_c = np.ascontiguousarray(
            xb.reshape(TC, P, D).transpose(1, 0, 2)
        ).astype(np.float32)
        x0bf_c = x0_c.astype(BF16)
        x0T_c = np.ascontiguousarray(
            xb.T.reshape(DT, P, T).transpose(1, 0, 2)
        ).astype(BF16)

        per_core.append(
            {
                "x0": x0_c,
                "x0bf": x0bf_c,
                "x0T": x0T_c,
                "enc": enc_h,
                "encv": encv_h,
                "dec": dec_h,
                "cosb": cos_h,
                "sinb": sin_h,
                "maskb": mask_h,
                "lm": lm_h,
            }
        )
    return per_core


def _get_nc():
    if "nc" not in _CACHE:
        _CACHE["nc"] = _build_bass()
    return _CACHE["nc"]


def kernel(idx, embed_w, encoder, encoder_v, decoder, lm_head, **extra):
    idx = np.asarray(idx)
    embed_w = np.asarray(embed_w, dtype=np.float32)
    encoder = np.asarray(encoder, dtype=np.float32)
    encoder_v = np.asarray(encoder_v, dtype=np.float32)
    decoder = np.asarray(decoder, dtype=np.float32)
    lm_head = np.asarray(lm_head, dtype=np.float32)

    nc = _get_nc()
    in_maps = _prep_inputs(idx, embed_w, encoder, encoder_v, decoder, lm_head)
    res = run_bass_kernel_spmd(nc, in_maps, core_ids=list(range(N_CORES)))
    _CACHE["last_results"] = res

    out = np.zeros((B, T, VOCAB), np.float32)
    for b in range(B):
        lg = res.results[b * NH]["logits"]  # [P, TC, VOCAB]
        out[b] = lg.transpose(1, 0, 2).reshape(T, VOCAB)
    return out


if __name__ == "__main__":
    rng = np.random.default_rng(0)
    ins = {
        "idx": rng.integers(0, VOCAB, (B, T)).astype(np.int32),
        "embed_w": (0.02 * rng.standard_normal((VOCAB, D))).astype(np.float32),
        "encoder": (0.02 * rng.standard_normal((NH, D, N))).astype(np.float32),
        "encoder_v": (0.02 * rng.standard_normal((NH, D, N))).astype(np.float32),
        "decoder": (0.02 * rng.standard_normal((NH * N, D))).astype(np.float32),
        "lm_head": (0.02 * rng.standard_normal((D, VOCAB))).astype(np.float32),
    }
    out = kernel(**ins)
    print("out", out.shape, out.dtype, float(np.abs(out).max()))

